# revision 3
# baseline (speedup 1.0000x reference)
"""DistMult+KBLN scoring kernel for 8 Trainium2 NeuronCores.

Math (eval mode, per reference):
    e1 = E[e1_idx]; r = R[r_idx]                       [B, D]
    score_l[b,e] = sum_d (e1*r)[b,d] * E[e,d]
    score_n[b,e] = sum_l nf[r_idx][b,l] * exp(-((n_h[b,l]-num_lit[e,l]-c[l])^2/var[l]))
    out = sigmoid(score_l + score_n)                   [B, E]

Key idea: the RBF factor phi(a - m) is a smooth 1-D Gaussian in the
num_lit value m, so expand it per-literal in a cubic B-spline basis on a
knot grid over m (spacing 0.5*sigma_l):

    phi(a[b,l] - m[e,l]) ~= sum_j c_j(a[b,l]) * B3((m[e,l]-xi_j)/h_l)

with c_j the least-squares coefficients (computed on host; max abs err
~2e-3).  Then

    score_n[b,e] = sum_{l,j} (w[b,l]*c_{l,j}) * Bmat[(l,j), e]

is ONE matmul with contraction dim K ~= 1800 (score_l's 200 E-rows are
appended to the same contraction).  The device kernel is a pure
streaming matmul: no on-device transcendentals at all, ~20 MB of basis
matrix per core streamed from HBM at full DMA bandwidth, accumulated in
PSUM (10 sub-accumulators as halves of 5 banks), sigmoid on the way out.

Sharding: entity axis split row-wise across 8 cores (5000 entities
each); host concatenates. No collectives.
"""
import sys

if "/opt/trn_rl_repo" not in sys.path:
    sys.path.insert(0, "/opt/trn_rl_repo")

import numpy as np

import concourse.bass as bass
import concourse.mybir as mybir
import concourse.tile as _tile
from concourse import tile
from concourse.bass_utils import run_bass_kernel_spmd
from concourse.vector_clock import ScopedClock

B = 64
NUM_ENT = 40000
NUM_REL = 1345
DIM = 200
N_LIT = 100
NCORES = 8
ESH = NUM_ENT // NCORES  # 5000 entities per core

SUBW = 500   # matmul free dim (one PSUM bank holds 512 fp32)
H = 0.5      # B-spline knot spacing in units of sigma_l

f32 = mybir.dt.float32
f16 = mybir.dt.float16
AF = mybir.ActivationFunctionType


def _drain_and_barrier_split(self, tick_clock, wait_clock):
    # This walrus build rejects >1 sync-wait per instruction; the tail Drain
    # normally carries one wait per active processor. Collect them on a probe
    # NOP instead (split later by _split_multi_waits) and emit a clean drain.
    nc = self.nc
    probe = nc.sync.nop(nofuse=True, hint="tail_wait_probe")
    wait_clock.add_sem_waits(probe.ins, ScopedClock({None: tick_clock.global_clock}))
    nc.sync.drain()
    nc.all_engine_barrier()
    assert self.sems is not None
    popped = nc._tile_sem_poison_stack.pop()
    assert popped is self._sem_poison
    nc.clear_and_free_semaphores(list(self.sems.allocated().values()))
    nc.all_engine_barrier()


_tile.TileContext._drain_and_barrier = _drain_and_barrier_split


def _split_multi_waits(nc: bass.Bass) -> int:
    """Hoist all-but-one sync wait from every instruction onto standalone
    single-wait EventSemaphore instructions inserted just before it (same
    engine, same block). Needed because this walrus build errors with
    "Too many sync wait commands" on instructions carrying >1 wait."""
    n_split = 0
    for bb in nc.m.functions[0].blocks:
        new_insts = []
        for inst in bb.instructions:
            waits = list(inst.sync_info.on_wait) if inst.sync_info else []
            if len(waits) > 1:
                for sw in waits[:-1]:
                    ev = mybir.InstEventSemaphore(
                        name=nc.get_next_instruction_name(),
                        engine=inst.engine,
                        ins=[],
                        outs=[],
                        sync_info=mybir.SyncInfo(on_wait=[sw], on_update=[]),
                    )
                    nc.register_instruction(ev)
                    new_insts.append(ev)
                    n_split += 1
                inst.sync_info.on_wait = waits[-1:]
            new_insts.append(inst)
        bb.instructions[:] = new_insts
    return n_split


def build_nc(kc: int) -> bass.Bass:
    """kc = number of 128-row contraction chunks."""
    nc = bass.Bass()
    K = kc * 128

    Bm_d = nc.dram_tensor("Bm", [K, ESH], f16, kind="ExternalInput")
    lhs_d = nc.dram_tensor("lhs", [128, kc * B], f16, kind="ExternalInput")
    out_d = nc.dram_tensor("out", [B, ESH], f32, kind="ExternalOutput")

    HALF = ESH // 2   # 2500
    NS = ESH // SUBW  # 10 sub-accumulators; 2 per PSUM bank (split partitions)

    with tile.TileContext(nc) as tc:
        with (
            tc.tile_pool(name="const", bufs=1) as cpool,
            tc.tile_pool(name="bm", bufs=3) as bmpool,
            tc.tile_pool(name="ps", bufs=1, space=bass.MemorySpace.PSUM) as pspool,
            tc.tile_pool(name="acc", bufs=1) as accpool,
        ):
            lhs_sb = cpool.tile([128, kc * B], f16, tag="lhs")
            warm = cpool.tile([128, 1], f32, tag="warm")
            out2 = accpool.tile([128, HALF], f32, tag="outsb")

            nc.sync.dma_start(lhs_sb[:], lhs_d[:])
            # load the Sigmoid act table while DMA streams (it's the only
            # table this kernel uses, so the tail pays no table switch)
            nc.scalar.activation(warm[:], warm[:], AF.Sigmoid)

            ps = [
                pspool.tile([128, SUBW], f32, tag=f"ps{s}", name=f"ps_{s}")
                for s in range(NS // 2)
            ]

            def acc_mm(s, lhsT, rhs, start, stop):
                bank, half = s % 5, s // 5
                nc.tensor.matmul(
                    ps[bank][half * B : (half + 1) * B, :], lhsT, rhs,
                    start=start, stop=stop, tile_position=(0, half * B),
                )

            for ck in range(kc):
                bt = bmpool.tile([128, ESH], f16, tag="bt")
                # split each chunk's 1.28 MB load across the three DGE paths
                nc.sync.dma_start(bt[0:43, :], Bm_d[ck * 128 : ck * 128 + 43, :])
                nc.scalar.dma_start(
                    bt[43:86, :], Bm_d[ck * 128 + 43 : ck * 128 + 86, :]
                )
                nc.gpsimd.dma_start(
                    bt[86:128, :], Bm_d[ck * 128 + 86 : (ck + 1) * 128, :]
                )
                s_order = (
                    (0, 5, 1, 6, 2, 7, 3, 8, 4, 9) if ck == kc - 1 else range(NS)
                )
                for s in s_order:
                    acc_mm(
                        s, lhs_sb[:, ck * B : (ck + 1) * B],
                        bt[:, s * SUBW : (s + 1) * SUBW],
                        start=(ck == 0), stop=(ck == kc - 1),
                    )

            # final sigmoid straight from PSUM, one full-width block per bank
            # (both partition-halves of a bank stop at adjacent matmuls)
            for bank in range(NS // 2):
                nc.scalar.activation(
                    out2[:, bank * SUBW : (bank + 1) * SUBW],
                    ps[bank][:],
                    AF.Sigmoid,
                )
            # rows 0:64 hold entities [0, 2500), rows 64:128 hold [2500, 5000);
            # quarter-granularity stores start as soon as their sigmoids land
            QW = 2 * SUBW + SUBW // 2  # 1250
            nc.sync.dma_start(out_d[:, 0:QW], out2[0:B, 0:QW])
            nc.scalar.dma_start(out_d[:, HALF : HALF + QW], out2[B:128, 0:QW])
            nc.sync.dma_start(out_d[:, QW:HALF], out2[0:B, QW:HALF])
            nc.scalar.dma_start(out_d[:, HALF + QW : ESH], out2[B:128, QW:HALF])

    _split_multi_waits(nc)
    return nc


def _bspline3(t):
    at = np.abs(t)
    r = np.zeros_like(at)
    m1 = at < 1
    r[m1] = (4 - 6 * at[m1] ** 2 + 3 * at[m1] ** 3) / 6
    m2 = (at >= 1) & (at < 2)
    r[m2] = ((2 - at[m2]) ** 3) / 6
    return r


def make_host_data(e1_idx, r_idx, E_weight, R_weight, num_lit, c, var, nf_weights):
    """Build the basis matrix [K, NUM_ENT] f16 and packed lhs [128, kc*B]."""
    e1_idx = np.asarray(e1_idx).astype(np.int64)
    r_idx = np.asarray(r_idx).astype(np.int64)
    E_weight = np.asarray(E_weight, dtype=np.float64)
    R_weight = np.asarray(R_weight, dtype=np.float64)
    num_lit = np.asarray(num_lit, dtype=np.float64)
    c = np.asarray(c, dtype=np.float64)
    var = np.asarray(var, dtype=np.float64)
    nf = np.asarray(nf_weights, dtype=np.float64)

    sig = np.sqrt(var)                  # [L]
    a_ctr = num_lit[e1_idx] - c         # [B, L] Gaussian centers, z units
    w = nf[r_idx]                       # [B, L]

    offs = np.zeros(N_LIT, dtype=np.int64)
    Js = np.zeros(N_LIT, dtype=np.int64)
    K = 0
    for l in range(N_LIT):
        J = int(np.floor(1.0 / (H * sig[l]))) + 6
        offs[l] = K
        Js[l] = J
        K += J
    K_phi = K
    kc = (K_phi + DIM + 127) // 128
    K_tot = kc * 128

    Bmat = np.zeros((K_tot, NUM_ENT), dtype=np.float16)
    lhsT = np.zeros((K_tot, B), dtype=np.float16)
    eidx = np.arange(NUM_ENT)
    for l in range(N_LIT):
        hz = H * sig[l]
        J = Js[l]
        off = offs[l]
        xi0 = -2 * hz
        t = (num_lit[:, l] - xi0) / hz
        j0 = np.floor(t).astype(np.int64)
        for k in range(4):
            j = j0 - 1 + k
            Bmat[off + j, eidx] = _bspline3(t - j).astype(np.float16)
        # least-squares spline coefficients for this literal's 64 centers
        zfit = np.linspace(-2 * hz, 1 + 2 * hz, 4 * J)
        xi = xi0 + hz * np.arange(J)
        Bz = _bspline3((zfit[:, None] - xi[None, :]) / hz)
        G = Bz.T @ Bz + 1e-9 * np.eye(J)
        S = np.exp(-(((a_ctr[:, l][:, None] - zfit[None, :]) / sig[l]) ** 2))
        C = np.linalg.solve(G, Bz.T @ S.T).T           # [B, J]
        lhsT[off : off + J, :] = (C * w[:, l][:, None]).T.astype(np.float16)
    # append the DistMult rows: score_l = (e1*r) @ E^T
    x = E_weight[e1_idx] * R_weight[r_idx]             # [B, D]
    Bmat[K_phi : K_phi + DIM, :] = E_weight.T.astype(np.float16)
    lhsT[K_phi : K_phi + DIM, :] = x.T.astype(np.float16)

    # pack lhs chunks: lhs_pack[p, ck*B + b] = lhsT[ck*128 + p, b]
    lhs_pack = np.ascontiguousarray(
        lhsT.reshape(kc, 128, B).transpose(1, 0, 2).reshape(128, kc * B)
    )
    return kc, Bmat, lhs_pack


def make_in_maps_from(kc, Bmat, lhs_pack):
    in_maps = []
    for core in range(NCORES):
        sl = slice(core * ESH, (core + 1) * ESH)
        in_maps.append(
            {
                "Bm": np.ascontiguousarray(Bmat[:, sl]),
                "lhs": lhs_pack,
            }
        )
    return in_maps


def make_in_maps(**inputs):
    kc, Bmat, lhs_pack = make_host_data(**inputs)
    return make_in_maps_from(kc, Bmat, lhs_pack)


_NC_CACHE = {}


def kernel(**inputs) -> np.ndarray:
    kc, Bmat, lhs_pack = make_host_data(**inputs)
    if kc not in _NC_CACHE:
        _NC_CACHE[kc] = build_nc(kc)
    nc = _NC_CACHE[kc]
    in_maps = make_in_maps_from(kc, Bmat, lhs_pack)
    res = run_bass_kernel_spmd(nc, in_maps, list(range(NCORES)))
    return np.concatenate([res.results[i]["out"] for i in range(NCORES)], axis=1)


# revision 5
# speedup vs baseline: 6.5744x; 6.5744x over previous
"""DistMult+KBLN scoring kernel for 8 Trainium2 NeuronCores.

Math (eval mode, per reference):
    e1 = E[e1_idx]; r = R[r_idx]                       [B, D]
    score_l[b,e] = sum_d (e1*r)[b,d] * E[e,d]
    score_n[b,e] = sum_l nf[r_idx][b,l] * exp(-((n_h[b,l]-num_lit[e,l]-c[l])^2/var[l]))
    out = sigmoid(score_l + score_n)                   [B, E]

Key idea: the RBF factor phi(a - m) is a smooth 1-D Gaussian in the
num_lit value m, so expand it per-literal in a cubic B-spline basis on a
knot grid over m (spacing 0.5*sigma_l):

    phi(a[b,l] - m[e,l]) ~= sum_j c_j(a[b,l]) * B3((m[e,l]-xi_j)/h_l)

with c_j the least-squares coefficients (computed on host; max abs err
~2e-3).  Then

    score_n[b,e] = sum_{l,j} (w[b,l]*c_{l,j}) * Bmat[(l,j), e]

is ONE matmul with contraction dim K ~= 1800 (score_l's 200 E-rows are
appended to the same contraction).  The device kernel is a pure
streaming matmul: no on-device transcendentals at all, ~20 MB of basis
matrix per core streamed from HBM at full DMA bandwidth, accumulated in
PSUM (10 sub-accumulators as halves of 5 banks), sigmoid on the way out.

Sharding: entity axis split row-wise across 8 cores (5000 entities
each); host concatenates. No collectives.
"""
import sys

if "/opt/trn_rl_repo" not in sys.path:
    sys.path.insert(0, "/opt/trn_rl_repo")

import numpy as np

import concourse.bass as bass
import concourse.mybir as mybir
import concourse.tile as _tile
from concourse import tile
from concourse.bass_utils import run_bass_kernel_spmd
from concourse.vector_clock import ScopedClock

B = 64
NUM_ENT = 40000
NUM_REL = 1345
DIM = 200
N_LIT = 100
NCORES = 8
ESH = NUM_ENT // NCORES  # 5000 entities per core

SUBW = 500   # matmul free dim (one PSUM bank holds 512 fp32)
H = 0.5      # B-spline knot spacing in units of sigma_l

f32 = mybir.dt.float32
f16 = mybir.dt.float16
AF = mybir.ActivationFunctionType


def _drain_and_barrier_split(self, tick_clock, wait_clock):
    # This walrus build rejects >1 sync-wait per instruction; the tail Drain
    # normally carries one wait per active processor. Collect them on a probe
    # NOP instead (split later by _split_multi_waits) and emit a clean drain.
    nc = self.nc
    probe = nc.sync.nop(nofuse=True, hint="tail_wait_probe")
    wait_clock.add_sem_waits(probe.ins, ScopedClock({None: tick_clock.global_clock}))
    nc.sync.drain()
    nc.all_engine_barrier()
    assert self.sems is not None
    popped = nc._tile_sem_poison_stack.pop()
    assert popped is self._sem_poison
    nc.clear_and_free_semaphores(list(self.sems.allocated().values()))
    nc.all_engine_barrier()


_tile.TileContext._drain_and_barrier = _drain_and_barrier_split


def _split_multi_waits(nc: bass.Bass) -> int:
    """Hoist all-but-one sync wait from every instruction onto standalone
    single-wait EventSemaphore instructions inserted just before it (same
    engine, same block). Needed because this walrus build errors with
    "Too many sync wait commands" on instructions carrying >1 wait."""
    n_split = 0
    for bb in nc.m.functions[0].blocks:
        new_insts = []
        for inst in bb.instructions:
            waits = list(inst.sync_info.on_wait) if inst.sync_info else []
            if len(waits) > 1:
                for sw in waits[:-1]:
                    ev = mybir.InstEventSemaphore(
                        name=nc.get_next_instruction_name(),
                        engine=inst.engine,
                        ins=[],
                        outs=[],
                        sync_info=mybir.SyncInfo(on_wait=[sw], on_update=[]),
                    )
                    nc.register_instruction(ev)
                    new_insts.append(ev)
                    n_split += 1
                inst.sync_info.on_wait = waits[-1:]
            new_insts.append(inst)
        bb.instructions[:] = new_insts
    return n_split


def build_nc(kc: int) -> bass.Bass:
    """kc = number of 128-row contraction chunks."""
    nc = bass.Bass()
    K = kc * 128

    Bm_d = nc.dram_tensor("Bm", [K, ESH], f16, kind="ExternalInput")
    lhs_d = nc.dram_tensor("lhs", [128, kc * B], f16, kind="ExternalInput")
    out_d = nc.dram_tensor("out", [B, ESH], f32, kind="ExternalOutput")

    HALF = ESH // 2   # 2500
    NS = ESH // SUBW  # 10 sub-accumulators; 2 per PSUM bank (split partitions)

    with tile.TileContext(nc) as tc:
        with (
            tc.tile_pool(name="const", bufs=1) as cpool,
            tc.tile_pool(name="bm", bufs=3) as bmpool,
            tc.tile_pool(name="ps", bufs=1, space=bass.MemorySpace.PSUM) as pspool,
            tc.tile_pool(name="acc", bufs=1) as accpool,
        ):
            lhs_sb = cpool.tile([128, kc * B], f16, tag="lhs")
            warm = cpool.tile([128, 1], f32, tag="warm")
            out2 = accpool.tile([128, HALF], f32, tag="outsb")

            nc.sync.dma_start(lhs_sb[:], lhs_d[:])
            # load the Sigmoid act table while DMA streams (it's the only
            # table this kernel uses, so the tail pays no table switch)
            nc.scalar.activation(warm[:], warm[:], AF.Sigmoid)

            ps = [
                pspool.tile([128, SUBW], f32, tag=f"ps{s}", name=f"ps_{s}")
                for s in range(NS // 2)
            ]

            def acc_mm(s, lhsT, rhs, start, stop):
                bank, half = s % 5, s // 5
                nc.tensor.matmul(
                    ps[bank][half * B : (half + 1) * B, :], lhsT, rhs,
                    start=start, stop=stop, tile_position=(0, half * B),
                )

            for ck in range(kc):
                bt = bmpool.tile([128, ESH], f16, tag="bt")
                # gpsimd = SWDGE: its descriptors spread across all 16 DMA
                # engines (the sync/scalar HWDGE queues serialize on one)
                nc.gpsimd.dma_start(bt[:], Bm_d[ck * 128 : (ck + 1) * 128, :])
                s_order = (
                    (0, 5, 1, 6, 2, 7, 3, 8, 4, 9) if ck == kc - 1 else range(NS)
                )
                for s in s_order:
                    acc_mm(
                        s, lhs_sb[:, ck * B : (ck + 1) * B],
                        bt[:, s * SUBW : (s + 1) * SUBW],
                        start=(ck == 0), stop=(ck == kc - 1),
                    )

            # final sigmoid straight from PSUM, one full-width block per bank
            # (both partition-halves of a bank stop at adjacent matmuls)
            for bank in range(NS // 2):
                nc.scalar.activation(
                    out2[:, bank * SUBW : (bank + 1) * SUBW],
                    ps[bank][:],
                    AF.Sigmoid,
                )
            # rows 0:64 hold entities [0, 2500), rows 64:128 hold [2500, 5000);
            # quarter-granularity stores start as soon as their sigmoids land
            QW = 2 * SUBW + SUBW // 2  # 1250
            nc.gpsimd.dma_start(out_d[:, 0:QW], out2[0:B, 0:QW])
            nc.gpsimd.dma_start(out_d[:, HALF : HALF + QW], out2[B:128, 0:QW])
            nc.gpsimd.dma_start(out_d[:, QW:HALF], out2[0:B, QW:HALF])
            nc.gpsimd.dma_start(out_d[:, HALF + QW : ESH], out2[B:128, QW:HALF])

    _split_multi_waits(nc)
    return nc


def _bspline3(t):
    at = np.abs(t)
    r = np.zeros_like(at)
    m1 = at < 1
    r[m1] = (4 - 6 * at[m1] ** 2 + 3 * at[m1] ** 3) / 6
    m2 = (at >= 1) & (at < 2)
    r[m2] = ((2 - at[m2]) ** 3) / 6
    return r


def make_host_data(e1_idx, r_idx, E_weight, R_weight, num_lit, c, var, nf_weights):
    """Build the basis matrix [K, NUM_ENT] f16 and packed lhs [128, kc*B]."""
    e1_idx = np.asarray(e1_idx).astype(np.int64)
    r_idx = np.asarray(r_idx).astype(np.int64)
    E_weight = np.asarray(E_weight, dtype=np.float64)
    R_weight = np.asarray(R_weight, dtype=np.float64)
    num_lit = np.asarray(num_lit, dtype=np.float64)
    c = np.asarray(c, dtype=np.float64)
    var = np.asarray(var, dtype=np.float64)
    nf = np.asarray(nf_weights, dtype=np.float64)

    sig = np.sqrt(var)                  # [L]
    a_ctr = num_lit[e1_idx] - c         # [B, L] Gaussian centers, z units
    w = nf[r_idx]                       # [B, L]

    offs = np.zeros(N_LIT, dtype=np.int64)
    Js = np.zeros(N_LIT, dtype=np.int64)
    K = 0
    for l in range(N_LIT):
        J = int(np.floor(1.0 / (H * sig[l]))) + 6
        offs[l] = K
        Js[l] = J
        K += J
    K_phi = K
    kc = (K_phi + DIM + 127) // 128
    K_tot = kc * 128

    Bmat = np.zeros((K_tot, NUM_ENT), dtype=np.float16)
    lhsT = np.zeros((K_tot, B), dtype=np.float16)
    eidx = np.arange(NUM_ENT)
    for l in range(N_LIT):
        hz = H * sig[l]
        J = Js[l]
        off = offs[l]
        xi0 = -2 * hz
        t = (num_lit[:, l] - xi0) / hz
        j0 = np.floor(t).astype(np.int64)
        for k in range(4):
            j = j0 - 1 + k
            Bmat[off + j, eidx] = _bspline3(t - j).astype(np.float16)
        # least-squares spline coefficients for this literal's 64 centers
        zfit = np.linspace(-2 * hz, 1 + 2 * hz, 4 * J)
        xi = xi0 + hz * np.arange(J)
        Bz = _bspline3((zfit[:, None] - xi[None, :]) / hz)
        G = Bz.T @ Bz + 1e-9 * np.eye(J)
        S = np.exp(-(((a_ctr[:, l][:, None] - zfit[None, :]) / sig[l]) ** 2))
        C = np.linalg.solve(G, Bz.T @ S.T).T           # [B, J]
        lhsT[off : off + J, :] = (C * w[:, l][:, None]).T.astype(np.float16)
    # append the DistMult rows: score_l = (e1*r) @ E^T
    x = E_weight[e1_idx] * R_weight[r_idx]             # [B, D]
    Bmat[K_phi : K_phi + DIM, :] = E_weight.T.astype(np.float16)
    lhsT[K_phi : K_phi + DIM, :] = x.T.astype(np.float16)

    # pack lhs chunks: lhs_pack[p, ck*B + b] = lhsT[ck*128 + p, b]
    lhs_pack = np.ascontiguousarray(
        lhsT.reshape(kc, 128, B).transpose(1, 0, 2).reshape(128, kc * B)
    )
    return kc, Bmat, lhs_pack


def make_in_maps_from(kc, Bmat, lhs_pack):
    in_maps = []
    for core in range(NCORES):
        sl = slice(core * ESH, (core + 1) * ESH)
        in_maps.append(
            {
                "Bm": np.ascontiguousarray(Bmat[:, sl]),
                "lhs": lhs_pack,
            }
        )
    return in_maps


def make_in_maps(**inputs):
    kc, Bmat, lhs_pack = make_host_data(**inputs)
    return make_in_maps_from(kc, Bmat, lhs_pack)


_NC_CACHE = {}


def kernel(**inputs) -> np.ndarray:
    kc, Bmat, lhs_pack = make_host_data(**inputs)
    if kc not in _NC_CACHE:
        _NC_CACHE[kc] = build_nc(kc)
    nc = _NC_CACHE[kc]
    in_maps = make_in_maps_from(kc, Bmat, lhs_pack)
    res = run_bass_kernel_spmd(nc, in_maps, list(range(NCORES)))
    return np.concatenate([res.results[i]["out"] for i in range(NCORES)], axis=1)


# revision 9
# speedup vs baseline: 7.4720x; 1.1365x over previous
"""DistMult+KBLN scoring kernel for 8 Trainium2 NeuronCores.

Math (eval mode, per reference):
    e1 = E[e1_idx]; r = R[r_idx]                       [B, D]
    score_l[b,e] = sum_d (e1*r)[b,d] * E[e,d]
    score_n[b,e] = sum_l nf[r_idx][b,l] * exp(-((n_h[b,l]-num_lit[e,l]-c[l])^2/var[l]))
    out = sigmoid(score_l + score_n)                   [B, E]

Key idea: the RBF factor phi(a - m) is a smooth 1-D Gaussian in the
num_lit value m, so expand it per-literal in a cubic B-spline basis on a
knot grid over m (spacing 0.5*sigma_l):

    phi(a[b,l] - m[e,l]) ~= sum_j c_j(a[b,l]) * B3((m[e,l]-xi_j)/h_l)

with c_j the least-squares coefficients (computed on host; max abs err
~2e-3).  Then

    score_n[b,e] = sum_{l,j} (w[b,l]*c_{l,j}) * Bmat[(l,j), e]

is ONE matmul with contraction dim K ~= 1800 (score_l's 200 E-rows are
appended to the same contraction).  The device kernel is a pure
streaming matmul: no on-device transcendentals at all, ~20 MB of basis
matrix per core streamed from HBM at full DMA bandwidth, accumulated in
PSUM (10 sub-accumulators as halves of 5 banks), sigmoid on the way out.

Sharding: entity axis split row-wise across 8 cores (5000 entities
each); host concatenates. No collectives.
"""
import sys

if "/opt/trn_rl_repo" not in sys.path:
    sys.path.insert(0, "/opt/trn_rl_repo")

import numpy as np

import concourse.bass as bass
import concourse.mybir as mybir
import concourse.tile as _tile
from concourse import tile
from concourse.bass_utils import run_bass_kernel_spmd
from concourse.vector_clock import ScopedClock

B = 64
NUM_ENT = 40000
NUM_REL = 1345
DIM = 200
N_LIT = 100
NCORES = 8
ESH = NUM_ENT // NCORES  # 5000 entities per core

SUBW = 500   # matmul free dim (one PSUM bank holds 512 fp32)
H = 0.6      # B-spline knot spacing in units of sigma_l

f32 = mybir.dt.float32
f16 = mybir.dt.float16
AF = mybir.ActivationFunctionType


def _drain_and_barrier_split(self, tick_clock, wait_clock):
    # This walrus build rejects >1 sync-wait per instruction; the tail Drain
    # normally carries one wait per active processor. Collect them on a probe
    # NOP instead (split later by _split_multi_waits) and emit a clean drain.
    nc = self.nc
    probe = nc.sync.nop(nofuse=True, hint="tail_wait_probe")
    wait_clock.add_sem_waits(probe.ins, ScopedClock({None: tick_clock.global_clock}))
    nc.sync.drain()
    nc.all_engine_barrier()
    assert self.sems is not None
    popped = nc._tile_sem_poison_stack.pop()
    assert popped is self._sem_poison
    nc.clear_and_free_semaphores(list(self.sems.allocated().values()))
    nc.all_engine_barrier()


_tile.TileContext._drain_and_barrier = _drain_and_barrier_split


def _split_multi_waits(nc: bass.Bass) -> int:
    """Hoist all-but-one sync wait from every instruction onto standalone
    single-wait EventSemaphore instructions inserted just before it (same
    engine, same block). Needed because this walrus build errors with
    "Too many sync wait commands" on instructions carrying >1 wait."""
    n_split = 0
    for bb in nc.m.functions[0].blocks:
        new_insts = []
        for inst in bb.instructions:
            waits = list(inst.sync_info.on_wait) if inst.sync_info else []
            if len(waits) > 1:
                for sw in waits[:-1]:
                    ev = mybir.InstEventSemaphore(
                        name=nc.get_next_instruction_name(),
                        engine=inst.engine,
                        ins=[],
                        outs=[],
                        sync_info=mybir.SyncInfo(on_wait=[sw], on_update=[]),
                    )
                    nc.register_instruction(ev)
                    new_insts.append(ev)
                    n_split += 1
                inst.sync_info.on_wait = waits[-1:]
            new_insts.append(inst)
        bb.instructions[:] = new_insts
    return n_split


def build_nc(kc: int) -> bass.Bass:
    """kc = number of 128-row contraction chunks."""
    nc = bass.Bass()
    K = kc * 128

    Bm_d = nc.dram_tensor("Bm", [K, ESH], f16, kind="ExternalInput")
    lhs_d = nc.dram_tensor("lhs", [128, kc * B], f16, kind="ExternalInput")
    out_d = nc.dram_tensor("out", [B, ESH], f32, kind="ExternalOutput")

    HALF = ESH // 2   # 2500
    NS = ESH // SUBW  # 10 sub-accumulators; 2 per PSUM bank (split partitions)

    with tile.TileContext(nc) as tc:
        with (
            tc.tile_pool(name="const", bufs=1) as cpool,
            tc.tile_pool(name="bm", bufs=4) as bmpool,
            tc.tile_pool(name="ps", bufs=1, space=bass.MemorySpace.PSUM) as pspool,
            tc.tile_pool(name="acc", bufs=1) as accpool,
        ):
            lhs_sb = cpool.tile([128, kc * B], f16, tag="lhs")
            warm = cpool.tile([128, 1], f32, tag="warm")
            out2 = accpool.tile([128, HALF], f32, tag="outsb")

            # lhs through SWDGE too (spread over 16 engines; it gates the
            # first matmul so it must land fast), issued before chunk 0
            nc.gpsimd.dma_start(lhs_sb[:], lhs_d[:])
            # load the Sigmoid act table while DMA streams (it's the only
            # table this kernel uses, so the tail pays no table switch)
            nc.scalar.activation(warm[:], warm[:], AF.Sigmoid)

            ps = [
                pspool.tile([128, SUBW], f32, tag=f"ps{s}", name=f"ps_{s}")
                for s in range(NS // 2)
            ]

            def acc_mm(s, lhsT, rhs, start, stop):
                bank, half = s % 5, s // 5
                nc.tensor.matmul(
                    ps[bank][half * B : (half + 1) * B, :], lhsT, rhs,
                    start=start, stop=stop, tile_position=(0, half * B),
                )

            for ck in range(kc):
                bt = bmpool.tile([128, ESH], f16, tag="bt")
                # gpsimd = SWDGE: its descriptors spread across all 16 DMA
                # engines (the sync/scalar HWDGE queues serialize on one)
                nc.gpsimd.dma_start(bt[:], Bm_d[ck * 128 : (ck + 1) * 128, :])
                s_order = (
                    (0, 5, 1, 6, 2, 7, 3, 8, 4, 9) if ck == kc - 1 else range(NS)
                )
                for s in s_order:
                    acc_mm(
                        s, lhs_sb[:, ck * B : (ck + 1) * B],
                        bt[:, s * SUBW : (s + 1) * SUBW],
                        start=(ck == 0), stop=(ck == kc - 1),
                    )

            # final sigmoid straight from PSUM, one full-width block per bank
            # (both partition-halves of a bank stop at adjacent matmuls)
            for bank in range(NS // 2):
                nc.scalar.activation(
                    out2[:, bank * SUBW : (bank + 1) * SUBW],
                    ps[bank][:],
                    AF.Sigmoid,
                )
            # rows 0:64 hold entities [0, 2500), rows 64:128 hold [2500, 5000);
            # quarter-granularity stores start as soon as their sigmoids land
            QW = 2 * SUBW + SUBW // 2  # 1250
            nc.gpsimd.dma_start(out_d[:, 0:QW], out2[0:B, 0:QW])
            nc.gpsimd.dma_start(out_d[:, HALF : HALF + QW], out2[B:128, 0:QW])
            nc.gpsimd.dma_start(out_d[:, QW:HALF], out2[0:B, QW:HALF])
            nc.gpsimd.dma_start(out_d[:, HALF + QW : ESH], out2[B:128, QW:HALF])

    _split_multi_waits(nc)
    return nc


def _bspline3(t):
    at = np.abs(t)
    r = np.zeros_like(at)
    m1 = at < 1
    r[m1] = (4 - 6 * at[m1] ** 2 + 3 * at[m1] ** 3) / 6
    m2 = (at >= 1) & (at < 2)
    r[m2] = ((2 - at[m2]) ** 3) / 6
    return r


def make_host_data(e1_idx, r_idx, E_weight, R_weight, num_lit, c, var, nf_weights):
    """Build the basis matrix [K, NUM_ENT] f16 and packed lhs [128, kc*B]."""
    e1_idx = np.asarray(e1_idx).astype(np.int64)
    r_idx = np.asarray(r_idx).astype(np.int64)
    E_weight = np.asarray(E_weight, dtype=np.float64)
    R_weight = np.asarray(R_weight, dtype=np.float64)
    num_lit = np.asarray(num_lit, dtype=np.float64)
    c = np.asarray(c, dtype=np.float64)
    var = np.asarray(var, dtype=np.float64)
    nf = np.asarray(nf_weights, dtype=np.float64)

    sig = np.sqrt(var)                  # [L]
    a_ctr = num_lit[e1_idx] - c         # [B, L] Gaussian centers, z units
    w = nf[r_idx]                       # [B, L]

    offs = np.zeros(N_LIT, dtype=np.int64)
    Js = np.zeros(N_LIT, dtype=np.int64)
    K = 0
    for l in range(N_LIT):
        J = int(np.ceil(1.0 / (H * sig[l]))) + 5
        offs[l] = K
        Js[l] = J
        K += J
    K_phi = K
    kc = (K_phi + DIM + 127) // 128
    K_tot = kc * 128

    Bmat = np.zeros((K_tot, NUM_ENT), dtype=np.float16)
    lhsT = np.zeros((K_tot, B), dtype=np.float16)
    eidx = np.arange(NUM_ENT)
    for l in range(N_LIT):
        hz = H * sig[l]
        J = Js[l]
        off = offs[l]
        xi0 = -2 * hz
        t = (num_lit[:, l] - xi0) / hz
        j0 = np.floor(t).astype(np.int64)
        for k in range(4):
            j = j0 - 1 + k
            Bmat[off + j, eidx] = _bspline3(t - j).astype(np.float16)
        # least-squares spline coefficients for this literal's 64 centers
        zfit = np.linspace(-2 * hz, 1 + 2 * hz, 4 * J)
        xi = xi0 + hz * np.arange(J)
        Bz = _bspline3((zfit[:, None] - xi[None, :]) / hz)
        G = Bz.T @ Bz + 1e-9 * np.eye(J)
        S = np.exp(-(((a_ctr[:, l][:, None] - zfit[None, :]) / sig[l]) ** 2))
        C = np.linalg.solve(G, Bz.T @ S.T).T           # [B, J]
        lhsT[off : off + J, :] = (C * w[:, l][:, None]).T.astype(np.float16)
    # append the DistMult rows: score_l = (e1*r) @ E^T
    x = E_weight[e1_idx] * R_weight[r_idx]             # [B, D]
    Bmat[K_phi : K_phi + DIM, :] = E_weight.T.astype(np.float16)
    lhsT[K_phi : K_phi + DIM, :] = x.T.astype(np.float16)

    # pack lhs chunks: lhs_pack[p, ck*B + b] = lhsT[ck*128 + p, b]
    lhs_pack = np.ascontiguousarray(
        lhsT.reshape(kc, 128, B).transpose(1, 0, 2).reshape(128, kc * B)
    )
    return kc, Bmat, lhs_pack


def make_in_maps_from(kc, Bmat, lhs_pack):
    in_maps = []
    for core in range(NCORES):
        sl = slice(core * ESH, (core + 1) * ESH)
        in_maps.append(
            {
                "Bm": np.ascontiguousarray(Bmat[:, sl]),
                "lhs": lhs_pack,
            }
        )
    return in_maps


def make_in_maps(**inputs):
    kc, Bmat, lhs_pack = make_host_data(**inputs)
    return make_in_maps_from(kc, Bmat, lhs_pack)


_NC_CACHE = {}


def kernel(**inputs) -> np.ndarray:
    kc, Bmat, lhs_pack = make_host_data(**inputs)
    if kc not in _NC_CACHE:
        _NC_CACHE[kc] = build_nc(kc)
    nc = _NC_CACHE[kc]
    in_maps = make_in_maps_from(kc, Bmat, lhs_pack)
    res = run_bass_kernel_spmd(nc, in_maps, list(range(NCORES)))
    return np.concatenate([res.results[i]["out"] for i in range(NCORES)], axis=1)
